# revision 8
# baseline (speedup 1.0000x reference)
"""BiAttention (BiDAF-style) Trainium2 Bass kernel.

doc:   [B=32, C=256, D=1024] f32
query: [B=32, C=256, Q=128]  f32
W:     [768] f32  (w_d | w_q | w_s, each [C])

sim[b,q,d] = dotd[b,d] + dotq[b,q] + sum_c w_s[c]*query[b,c,q]*doc[b,c,d]
s1 = softmax_q(sim)   s2 = softmax_d(sim)
d_d2q = query @ s1              [C,D]
q_q2d = doc @ s2.T              [C,Q]
d_q2d = q_q2d @ s1              [C,D]
q_d2q = query @ (s1 @ s2.T)     [C,Q]   (Q x Q gram trick)
d_out = [doc, d_d2q, doc*d_d2q, doc*d_q2d]      [4C, D]
q_out = [query, q_q2d, query*q_q2d, query*q_d2q][4C, Q]

Sharding: data-parallel over batch, 4 batches per core on 8 cores.

Layout: sim/E in [q=128part, d=1024free]; exp() once (values are O(+-8) so no
max-subtraction needed); s1 obtained by per-partition normalize of transposed
E; s2's row normalizer (1/rowsum E) folded as per-partition scale into the
outputs of matmuls that contract with E^T.
"""

import numpy as np

import concourse.bass as bass
import concourse.tile as tile
from concourse import bacc, masks, mybir
from concourse.bass_utils import run_bass_kernel_spmd

B, C, D, Q = 32, 256, 1024, 128
NCORES = 8
BL = B // NCORES  # batches per core
FP32 = mybir.dt.float32
AF = mybir.ActivationFunctionType
NH = D // 512  # psum-bank halves of D
NT = D // 128  # 128-wide tiles of D


def _emit(ctx, tc, dout_h, qout_h, doc_h, query_h, w_h):
    nc = tc.nc

    const = ctx.enter_context(tc.tile_pool(name="const", bufs=1))
    ident = const.tile([128, 128], FP32)
    masks.make_identity(nc, ident[:])
    # wcols[:, g*2+j][p] = W[g*256 + j*128 + p];  g: 0=w_d 1=w_q 2=w_s
    wcols = const.tile([128, 6], FP32)
    nc.sync.dma_start(wcols[:], w_h.rearrange("(g j p) -> p (g j)", p=128, j=2))
    ones_row = const.tile([1, D], FP32)
    nc.gpsimd.memset(ones_row[:], 1.0)

    io = ctx.enter_context(tc.tile_pool(name="io", bufs=2))
    wk = ctx.enter_context(tc.tile_pool(name="wk", bufs=2))
    st = ctx.enter_context(tc.tile_pool(name="st", bufs=4))
    # PSUM: sim keeps 2 banks; everything else rotates through 6 one-bank slots
    psA = ctx.enter_context(tc.tile_pool(name="psA", bufs=1, space="PSUM"))
    psS = ctx.enter_context(tc.tile_pool(name="psS", bufs=6, space="PSUM"))

    def ps(shape, name):
        return psS.tile(shape, FP32, tag="s", name=name)

    # alternate PSUM->SBUF copies between the two engines that can read PSUM
    def cp(i, out, in_):
        if i % 2 == 0:
            nc.scalar.copy(out, in_)
        else:
            nc.vector.tensor_copy(out, in_)

    for b in range(BL):
        # ---- loads --------------------------------------------------------
        dc = io.tile([128, 2, D], FP32)  # dc[p,j,:] = doc[b, j*128+p, :]
        nc.sync.dma_start(dc[:], doc_h[b].rearrange("(j p) d -> p j d", p=128))
        qr = io.tile([128, 2, Q], FP32)
        nc.sync.dma_start(qr[:], query_h[b].rearrange("(j p) q -> p j q", p=128))

        # ---- sim = (w_s*query)^T doc + 1*dotd + dotq*1 --------------------
        wsq = wk.tile([128, 2, Q], FP32)
        for j in range(2):
            nc.vector.tensor_scalar_mul(wsq[:, j, :], qr[:, j, :], wcols[:, 4 + j : 5 + j])

        dotq_ps = ps([1, Q], "dotq_ps")
        for j in range(2):
            nc.tensor.matmul(dotq_ps[:], wcols[:, 2 + j : 3 + j], qr[:, j, :],
                             start=(j == 0), stop=(j == 1))
        dd_ps = [ps([1, 512], f"dotd_ps{h}") for h in range(NH)]
        for h in range(NH):
            for j in range(2):
                nc.tensor.matmul(dd_ps[h][:], wcols[:, j : j + 1],
                                 dc[:, j, h * 512 : (h + 1) * 512],
                                 start=(j == 0), stop=(j == 1))
        dotq_s = wk.tile([1, Q], FP32)
        nc.scalar.copy(dotq_s[:], dotq_ps[:])
        dotd_s = wk.tile([1, D], FP32)
        for h in range(NH):
            nc.scalar.copy(dotd_s[:, h * 512 : (h + 1) * 512], dd_ps[h][:])

        sim_ps = psA.tile([128, D], FP32)
        for h in range(NH):
            sl = slice(h * 512, (h + 1) * 512)
            nc.tensor.matmul(sim_ps[:, sl], wsq[:, 0, :], dc[:, 0, sl],
                             start=True, stop=False)
            nc.tensor.matmul(sim_ps[:, sl], wsq[:, 1, :], dc[:, 1, sl],
                             start=False, stop=False)
            # rank-1 terms: + 1*dotd[d]  and  + dotq[q]*1
            nc.tensor.matmul(sim_ps[:, sl], ones_row[:, :Q], dotd_s[:, sl],
                             start=False, stop=False)
            nc.tensor.matmul(sim_ps[:, sl], dotq_s[:], ones_row[:, sl],
                             start=False, stop=True)

        # ---- E = exp(sim), row sums, transposes ---------------------------
        E = wk.tile([128, D], FP32)
        rsq = wk.tile([128, 1], FP32)
        nc.scalar.activation(E[:], sim_ps[:], AF.Exp, accum_out=rsq[:])
        rsqr = wk.tile([128, 1], FP32)
        nc.vector.reciprocal(rsqr[:], rsq[:])

        ET = wk.tile([128, NT, 128], FP32)  # ET[:,t,:] = E[:, t-tile].T
        for t in range(NT):
            tp = ps([128, 128], "tpE")
            nc.tensor.transpose(tp[:], E[:, t * 128 : (t + 1) * 128], ident[:])
            cp(t, ET[:, t, :], tp[:])

        rsd = wk.tile([128, NT], FP32)
        for t in range(NT):
            nc.vector.reduce_sum(rsd[:, t : t + 1], ET[:, t, :], axis=mybir.AxisListType.X)
        rsdr = wk.tile([128, NT], FP32)
        nc.vector.reciprocal(rsdr[:], rsd[:])

        S1T = wk.tile([128, NT, 128], FP32)  # s1 transposed: [d, q]
        for t in range(NT):
            nc.vector.tensor_scalar_mul(S1T[:, t, :], ET[:, t, :], rsdr[:, t : t + 1])
        S1 = wk.tile([128, D], FP32)  # s1: [q, d]
        for t in range(NT):
            tp = ps([128, 128], "tpS1")
            nc.tensor.transpose(tp[:], S1T[:, t, :], ident[:])
            cp(t, S1[:, t * 128 : (t + 1) * 128], tp[:])

        qT = wk.tile([128, C], FP32)  # query^T: [q, c]
        for j in range(2):
            tp = ps([128, 128], "tpQ")
            nc.tensor.transpose(tp[:], qr[:, j, :], ident[:])
            cp(j, qT[:, j * 128 : (j + 1) * 128], tp[:])

        dT = wk.tile([128, NT, C], FP32)  # doc^T tiles: dT[:,t,:] = [d_t, c]
        for t in range(NT):
            for j in range(2):
                tp = ps([128, 128], "tpD")
                nc.tensor.transpose(tp[:], dc[:, j, t * 128 : (t + 1) * 128], ident[:])
                cp(t * 2 + j, dT[:, t, j * 128 : (j + 1) * 128], tp[:])

        # ---- gram M2u[q1,q2] = sum_d S1[q1,d] E[q2,d] ---------------------
        m2_ps = ps([128, 128], "m2_ps")
        for t in range(NT):
            nc.tensor.matmul(m2_ps[:], S1T[:, t, :], ET[:, t, :],
                             start=(t == 0), stop=(t == NT - 1))
        m2 = wk.tile([128, 128], FP32)
        nc.scalar.copy(m2[:], m2_ps[:])

        # ---- Tq2d[q,c] = q_q2d^T = rowscale( sum_d E[q,d] doc[c,d] ) ------
        tq_ps = ps([128, C], "tq_ps")
        for t in range(NT):
            nc.tensor.matmul(tq_ps[:], ET[:, t, :], dT[:, t, :],
                             start=(t == 0), stop=(t == NT - 1))
        Tq2d = wk.tile([128, C], FP32)
        nc.vector.tensor_scalar_mul(Tq2d[:], tq_ps[:], rsqr[:])

        # ---- q_d2q^T[q2,c] = rowscale( sum_q1 M2u[q1,q2] qT[q1,c] ) -------
        qd_ps = ps([128, C], "qd_ps")
        nc.tensor.matmul(qd_ps[:], m2[:], qT[:], start=True, stop=True)
        qd2qT = wk.tile([128, C], FP32)
        nc.vector.tensor_scalar_mul(qd2qT[:], qd_ps[:], rsqr[:])

        # ---- d-side outputs ----------------------------------------------
        for j in range(2):
            nc.sync.dma_start(dout_h[b, j * 128 : (j + 1) * 128, :], dc[:, j, :])
        for j in range(2):
            for h in range(NH):
                sl = slice(h * 512, (h + 1) * 512)
                rows = slice(C + j * 128, C + (j + 1) * 128)
                p1 = ps([128, 512], "dd2q_ps")
                nc.tensor.matmul(p1[:], qT[:, j * 128 : (j + 1) * 128], S1[:, sl],
                                 start=True, stop=True)
                raw = st.tile([128, 512], FP32, name="dd_raw")
                nc.scalar.copy(raw[:], p1[:])
                nc.sync.dma_start(dout_h[b, rows, sl], raw[:])
                prod = st.tile([128, 512], FP32, name="dd_prod")
                nc.vector.tensor_mul(prod[:], dc[:, j, sl], p1[:])
                nc.sync.dma_start(dout_h[b, C * 2 + j * 128 : C * 2 + (j + 1) * 128, sl], prod[:])

                p2 = ps([128, 512], "dq2d_ps")
                nc.tensor.matmul(p2[:], Tq2d[:, j * 128 : (j + 1) * 128], S1[:, sl],
                                 start=True, stop=True)
                prod2 = st.tile([128, 512], FP32, name="dq_prod")
                nc.vector.tensor_mul(prod2[:], dc[:, j, sl], p2[:])
                nc.sync.dma_start(dout_h[b, C * 3 + j * 128 : C * 3 + (j + 1) * 128, sl], prod2[:])

        # ---- q-side outputs ----------------------------------------------
        for j in range(2):
            nc.sync.dma_start(qout_h[b, j * 128 : (j + 1) * 128, :], qr[:, j, :])
            tp = ps([128, 128], "tpQQ")
            nc.tensor.transpose(tp[:], Tq2d[:, j * 128 : (j + 1) * 128], ident[:])
            raw = st.tile([128, 128], FP32, name="qq_raw")
            nc.scalar.copy(raw[:], tp[:])
            nc.sync.dma_start(qout_h[b, C + j * 128 : C + (j + 1) * 128, :], raw[:])
            prod = st.tile([128, 128], FP32, name="qq_prod")
            nc.vector.tensor_mul(prod[:], qr[:, j, :], tp[:])
            nc.sync.dma_start(qout_h[b, C * 2 + j * 128 : C * 2 + (j + 1) * 128, :], prod[:])

            tp2 = ps([128, 128], "tpQD")
            nc.tensor.transpose(tp2[:], qd2qT[:, j * 128 : (j + 1) * 128], ident[:])
            prod2 = st.tile([128, 128], FP32, name="qd_prod")
            nc.vector.tensor_mul(prod2[:], qr[:, j, :], tp2[:])
            nc.sync.dma_start(qout_h[b, C * 3 + j * 128 : C * 3 + (j + 1) * 128, :], prod2[:])


_BUILT = None


def build():
    """Build + compile the Bass module once per process."""
    global _BUILT
    if _BUILT is not None:
        return _BUILT
    nc = bacc.Bacc("TRN2", target_bir_lowering=False, debug=False,
                   num_devices=NCORES)
    doc_h = nc.dram_tensor("doc", [BL, C, D], FP32, kind="ExternalInput").ap()
    query_h = nc.dram_tensor("query", [BL, C, Q], FP32, kind="ExternalInput").ap()
    w_h = nc.dram_tensor("W", [3 * C], FP32, kind="ExternalInput").ap()
    dout_h = nc.dram_tensor("d_out", [BL, 4 * C, D], FP32, kind="ExternalOutput").ap()
    qout_h = nc.dram_tensor("q_out", [BL, 4 * C, Q], FP32, kind="ExternalOutput").ap()

    from contextlib import ExitStack
    with tile.TileContext(nc) as tc:
        with ExitStack() as ctx:
            _emit(ctx, tc, dout_h, qout_h, doc_h, query_h, w_h)
    nc.compile()
    _BUILT = nc
    return nc


def make_in_maps(doc, query, W):
    return [
        {"doc": np.ascontiguousarray(doc[c * BL : (c + 1) * BL]),
         "query": np.ascontiguousarray(query[c * BL : (c + 1) * BL]),
         "W": np.asarray(W)}
        for c in range(NCORES)
    ]


def kernel(doc, query, W):
    doc = np.asarray(doc, dtype=np.float32)
    query = np.asarray(query, dtype=np.float32)
    W = np.asarray(W, dtype=np.float32)
    nc = build()
    res = run_bass_kernel_spmd(nc, make_in_maps(doc, query, W),
                               core_ids=list(range(NCORES)))
    outs = res.results
    d_full = np.concatenate([outs[c]["d_out"] for c in range(NCORES)], axis=0)
    q_full = np.concatenate([outs[c]["q_out"] for c in range(NCORES)], axis=0)
    return (d_full, q_full)
